# revision 26
# baseline (speedup 1.0000x reference)
"""Antisymmetric RNN kernel for Trainium2, data-parallel over batch on 8 cores.

Math (reference):
    M = W - W^T - gamma*I
    h_t = x_t @ V + bias                      [B, U]
    state_{t+1} = state_t + eps*tanh(h_t + state_t @ M)
    out[:, t] = state_{t+1}

Device formulation (per core, B_local=16), rescaled S' = state/eps,
M' = eps*M:
    S'_{t+1} = S'_t + tanh(h_t + S'_t @ M')

||M'|| is tiny (diag -1e-4, off-diag ~5e-7), so one Picard sweep over
the whole trajectory with a coarse (32-step piecewise-constant) state
estimate in the correction term converges:
    Sc  = x0/eps + prefix sums of 32-step block sums of h
          (tanh(h) ~ h there: the cubic error is zero-mean and enters
          z only through the 1e-4-scaled M')
    S   = x0/eps + cumsum(tanh(h + Sc@M'))   [exact, full res]
Measured rel err vs the exact recurrence: ~6e-3 (threshold 2e-2).

The 32-step block sums of x are computed on the host (input prep), so
sweep 0 on device is just 2 matmuls (xblk @ V) + 32 short prefix scans.
DVE's serial scan runs at ~2.4 cyc/element, so the full-res cumsum is
4-way de-interleaved: quad sums Q scan at T/4 resolution (f32), and
S[4k+c] are reconstructed with single adds off the f32 carries.

Layout: partitions carry u (2 chunks of 128); free dims are
(batch-outer, time-inner); th tile is [.., kc, 4] (par minor) so ACT
writes stay packed and parity slices are strided DVE reads. The coarse
Sc feeds PE matmuls via stride-0 broadcast APs (explicit dep edge -
broadcast reads are invisible to tile dep tracking).
"""

import sys

sys.path.insert(0, "/opt/trn_rl_repo")

import numpy as np
import ml_dtypes

import concourse.bass as bass
import concourse.bacc as bacc
import concourse.mybir as mybir
import concourse.tile as tile
from concourse.tile import add_dep_helper

EPS = 0.01
GAMMA = 0.01
B, T, D, U = 128, 1024, 128, 256
NCORES = 8
BL = B // NCORES  # 16 batch rows per core
NK = U // 128  # 2 u-chunks
W32 = NK * BL  # 32 (chunk, batch) columns
TCB = 128  # timesteps per PSUM tile / ACT instruction (4 banks)
QB = 4  # batch rows per matmul accumulation group (1 bank)
RB = 32  # coarse-S0 block size (piecewise-constant correction)

F32 = mybir.dt.float32
BF16 = mybir.dt.bfloat16
BF16_NP = ml_dtypes.bfloat16

_CACHED = {}


def build_nc(t_steps=T):
    nc = bacc.Bacc(None, target_bir_lowering=False)
    x_d = nc.declare_dram_parameter("xT", [D, BL, t_steps], BF16, isOutput=False)
    m_d = nc.declare_dram_parameter("Mp", [128, NK, NK, 128], BF16, isOutput=False)
    v_d = nc.declare_dram_parameter("Vp", [D, NK, 128], BF16, isOutput=False)
    b_d = nc.declare_dram_parameter("bT", [128, NK], F32, isOutput=False)
    x0_d = nc.declare_dram_parameter("x0T", [128, NK], F32, isOutput=False)
    x0b_d = nc.declare_dram_parameter("x0B", [128, W32], BF16, isOutput=False)
    x0f_d = nc.declare_dram_parameter("x0F", [128, W32], F32, isOutput=False)
    nb = t_steps // RB  # number of coarse blocks
    xb_d = nc.declare_dram_parameter("xB", [D, BL, nb], BF16, isOutput=False)
    nq = t_steps // 4
    ch1 = min(512, t_steps)
    n1 = t_steps // ch1
    kc = ch1 // 4
    o3_d = nc.declare_dram_parameter("out3", [128, W32, nq], F32, isOutput=True)
    o012_d = nc.declare_dram_parameter(
        "out012", [128, W32, n1, 3, kc], BF16, isOutput=True
    )

    Tanh = mybir.ActivationFunctionType.Tanh
    ADD = mybir.AluOpType.add
    BYPASS = mybir.AluOpType.bypass

    tcb = min(TCB, ch1)
    assert t_steps % ch1 == 0 and ch1 % tcb == 0 and tcb % RB == 0

    with tile.TileContext(nc) as tc:
        with (
            tc.tile_pool(name="const", bufs=1) as cpool,
            tc.tile_pool(name="xp", bufs=1) as xpool,
            tc.tile_pool(name="th", bufs=2) as thpool,
            tc.tile_pool(name="rec", bufs=1) as rpool,
            tc.tile_pool(name="ps", bufs=1, space=bass.MemorySpace.PSUM) as ppool,
        ):
            m_sb = cpool.tile([128, NK, NK, 128], BF16)
            v_sb = cpool.tile([D, NK, 128], BF16)
            b_sb = cpool.tile([128, NK], F32)
            x0_sb = cpool.tile([128, NK], F32)
            xb_sb = cpool.tile([D, BL, nb], BF16)
            # coarse prefix sums; slot 0 = x0/eps, slot m = prefix thru block m-1
            sc_sb = cpool.tile([128, W32, 1 + nb], BF16)
            # quad prefix sums (S at t=4k+3); slot 0 = x0/eps
            sq_sb = cpool.tile([128, W32, 1 + nq], F32)
            nc.sync.dma_start(xb_sb[:], xb_d[:])
            nc.sync.dma_start(v_sb[:], v_d[:])
            nc.sync.dma_start(m_sb[:], m_d[:])
            nc.sync.dma_start(b_sb[:], b_d[:])
            nc.sync.dma_start(x0_sb[:], x0_d[:])
            x0b_dma = nc.sync.dma_start(sc_sb[:, :, 0:1], x0b_d[:].unsqueeze(2))
            nc.sync.dma_start(sq_sb[:, :, 0:1], x0f_d[:].unsqueeze(2))

            x_sb = xpool.tile([D, BL, t_steps], BF16)
            nx = max(1, t_steps // 128)
            for c in range(nx):
                sl = slice(c * (t_steps // nx), (c + 1) * (t_steps // nx))
                nc.sync.dma_start(x_sb[:, :, sl], x_d[:, :, sl])

            # ---- sweep 0: coarse Sc: bsum = xblk@V (PE), short prefix scans
            # bsum psum borrows bank 0 of the z tiles (flat [b, m] layout)
            tot = BL * nb
            rows = max(1, tot // 128)
            cols = tot // rows
            bs_tiles = []
            for h in range(NK):
                z = ppool.tile([128, BL, tcb], F32, tag=f"z{h}")
                bs = z[:, 0:rows, 0:cols].rearrange(
                    "p a (b m) -> p (a b) m", m=nb
                )
                nc.tensor.matmul(bs, v_sb[:, h, :], xb_sb[:], start=True, stop=True)
                bs_tiles.append(z)
            dummy = b_sb[:, 0:1].broadcast_to([128, nb])
            sc_scans = [None] * W32
            for h in range(NK):
                z = bs_tiles[h]
                for b in range(BL):
                    j = h * BL + b
                    fi = b * nb
                    sc_scans[j] = nc.vector.tensor_tensor_scan(
                        sc_sb[:, j, 1 : 1 + nb],
                        z[:, fi // 128, fi % 128 : fi % 128 + nb],
                        dummy,
                        x0_sb[:, h : h + 1],
                        ADD,
                        BYPASS,
                    ).ins
            # broadcast-AP reads are invisible to tile dep tracking: first
            # block gets per-column edges onto the Sc scans (so PE starts as
            # soon as the columns it needs are ready), the next block one
            # coarse edge onto the last scan; PE program order covers the rest
            state = {"blk": 0}

            def emit_block(th_out, t0, h):
                # z = x@V + Sc@M' in PSUM; th_out = tanh(z + b)
                z = ppool.tile([128, BL, tcb], F32, tag=f"z{h}")
                m0 = t0 // RB  # first coarse block of this range
                nblk = tcb // RB
                blk_i = state["blk"]
                state["blk"] += 1
                for q in range(BL // QB):
                    zq = z[:, q * QB : (q + 1) * QB, :]
                    xq = x_sb[:, q * QB : (q + 1) * QB, t0 : t0 + tcb]
                    nc.tensor.matmul(zq, v_sb[:, h, :], xq, start=True, stop=False)
                    for k in range(NK):
                        sq = (
                            sc_sb[
                                :,
                                k * BL + q * QB : k * BL + (q + 1) * QB,
                                m0 : m0 + nblk,
                            ]
                            .unsqueeze(3)
                            .broadcast_to([128, QB, nblk, RB])
                        )
                        mm = nc.tensor.matmul(
                            zq, m_sb[:, k, h, :], sq, start=False, stop=(k == NK - 1)
                        )
                        if blk_i == 0:
                            add_dep_helper(
                                mm.ins,
                                sc_scans[k * BL + q * QB + QB - 1],
                                reason="Sc broadcast read",
                            )
                            if q == 0 and k == 0:
                                add_dep_helper(
                                    mm.ins, x0b_dma.ins, reason="Sc slot0 read"
                                )
                        elif blk_i == 1 and q == 0 and k == 0:
                            add_dep_helper(
                                mm.ins, sc_scans[W32 - 1], reason="Sc broadcast read"
                            )
                nc.scalar.activation(th_out, z[:], Tanh, bias=b_sb[:, h : h + 1])

            # ---- sweep 1: full-res S via 4-way de-interleaved cumsum ----
            # th1 tile is [128, W32, kc, 4]: same memory as packed time order
            # (par minor), so ACT writes stay contiguous and parity slices
            # are strided reads on DVE.
            for c in range(n1):
                th = thpool.tile([128, W32, kc, 4], BF16, tag="th1")
                for blk in range(ch1 // tcb):
                    t0 = c * ch1 + blk * tcb
                    k0 = blk * (tcb // 4)
                    for h in range(NK):
                        out_ap = th[
                            :, h * BL : (h + 1) * BL, k0 : k0 + tcb // 4, :
                        ].rearrange("p b k r -> p b (k r)")
                        emit_block(out_ap, t0, h)
                # prep: pair sums (strided reads, 1x) then quad sums (2x);
                # p23 runs on the otherwise-idle GPSIMD in parallel with p01
                p01 = rpool.tile([128, W32, kc], BF16, tag="p01")
                p23 = rpool.tile([128, W32, kc], BF16, tag="p23")
                qq = rpool.tile([128, W32, kc], BF16, tag="qq")
                nc.gpsimd.tensor_tensor(p23[:], th[:, :, :, 2], th[:, :, :, 3], ADD)
                nc.vector.tensor_tensor(p01[:], th[:, :, :, 0], th[:, :, :, 1], ADD)
                nc.vector.tensor_tensor(qq[:], p01[:], p23[:], ADD)
                # scan quads (f32 state+out), chained through sq_sb slots
                for h in range(NK):
                    for b in range(BL):
                        j = h * BL + b
                        nc.vector.tensor_tensor_scan(
                            sq_sb[:, j, 1 + c * kc : 1 + (c + 1) * kc],
                            qq[:, j, :],
                            qq[:, j, :],
                            sq_sb[:, j, c * kc : c * kc + 1],
                            ADD,
                            BYPASS,
                        )
                nc.sync.dma_start(
                    o3_d[:, :, c * kc : (c + 1) * kc],
                    sq_sb[:, :, 1 + c * kc : 1 + (c + 1) * kc],
                )
                # reconstruct parities 0..2 off the f32 carries C:
                # S[4k]=C+q0, S[4k+1]=C+p01, S[4k+2]=S[4k+1]+q2
                recs = rpool.tile([128, W32, 3, kc], BF16, tag="recs")
                cq = sq_sb[:, :, c * kc : (c + 1) * kc]
                nc.gpsimd.tensor_tensor(recs[:, :, 0, :], cq, th[:, :, :, 0], ADD)
                nc.vector.tensor_tensor(recs[:, :, 1, :], cq, p01[:], ADD)
                nc.vector.tensor_tensor(
                    recs[:, :, 2, :], recs[:, :, 1, :], th[:, :, :, 2], ADD
                )
                nc.sync.dma_start(o012_d[:, :, c, :, :], recs[:])

    nc.compile()
    return nc


def _prep_consts(V, W, bias, x0):
    M = W - W.T - GAMMA * np.eye(U, dtype=np.float32)
    Mp = (EPS * M).reshape(NK, 128, NK, 128).transpose(1, 0, 2, 3)
    Vp = V.reshape(D, NK, 128)
    bT = np.ascontiguousarray(bias.reshape(NK, 128).T)
    x0T = np.ascontiguousarray((x0 / EPS).reshape(NK, 128).T)
    x0B = np.repeat(x0T, BL, axis=1)  # [128, W32] broadcast per (chunk, batch)
    return {
        "Mp": np.ascontiguousarray(Mp).astype(BF16_NP),
        "Vp": np.ascontiguousarray(Vp).astype(BF16_NP),
        "bT": bT.astype(np.float32),
        "x0T": x0T.astype(np.float32),
        "x0B": np.ascontiguousarray(x0B).astype(BF16_NP),
        "x0F": np.ascontiguousarray(x0B).astype(np.float32),
    }


def _install_ntff_hook():
    # Register the axon NTFF profile hook if the image's antenv lacks it,
    # so trace=True can return exec_time_ns. Harmless if anything fails.
    import types

    try:
        import antenv.axon_hooks  # noqa: F401

        return
    except ImportError:
        pass
    try:
        import antenv
        from trn_agent_boot.trn_boot import _ntff_profile_via_ctypes

        mod = types.ModuleType("antenv.axon_hooks")
        _h = [None]
        mod.set_axon_ntff_profile_hook = lambda h: _h.__setitem__(0, h)
        mod.get_axon_ntff_profile_hook = lambda: _h[0]
        sys.modules["antenv.axon_hooks"] = mod
        antenv.axon_hooks = mod
        mod.set_axon_ntff_profile_hook(
            _ntff_profile_via_ctypes("/opt/axon/libaxon_pjrt.so")
        )
    except Exception:
        pass


def kernel(inputs, V, W, bias, x0, _t_steps=None, _trace=False):
    _install_ntff_hook()
    from concourse.bass_utils import run_bass_kernel_spmd

    inputs = np.asarray(inputs, dtype=np.float32)
    V = np.asarray(V, dtype=np.float32)
    W = np.asarray(W, dtype=np.float32)
    bias = np.asarray(bias, dtype=np.float32)
    x0 = np.asarray(x0, dtype=np.float32)

    t_steps = _t_steps or inputs.shape[1]
    key = t_steps
    if key not in _CACHED:
        _CACHED[key] = build_nc(t_steps)
    nc = _CACHED[key]

    consts = _prep_consts(V, W, bias, x0)
    nb = t_steps // RB
    in_maps = []
    for i in range(NCORES):
        shard = inputs[i * BL : (i + 1) * BL, :t_steps, :]  # [16, t, 128]
        xT = np.ascontiguousarray(shard.transpose(2, 0, 1)).astype(BF16_NP)
        xblk = shard.reshape(BL, nb, RB, D).sum(axis=2)  # [16, nb, 128] f32
        xB = np.ascontiguousarray(xblk.transpose(2, 0, 1)).astype(BF16_NP)
        in_maps.append({"xT": xT, "xB": xB, **consts})

    res = run_bass_kernel_spmd(nc, in_maps, list(range(NCORES)), trace=_trace)
    nq = t_steps // 4
    outs = []
    for i in range(NCORES):
        o3 = res.results[i]["out3"]  # [128, W32, nq] f32 (parity 3)
        o012 = res.results[i]["out012"].astype(np.float32)  # [128,W32,n1,3,kc]
        n1 = o012.shape[2]
        # S[p, j=(h,b), par, k] -> out[b, 4k+par, h*128+p]
        S = np.empty((128, W32, 4, nq), dtype=np.float32)
        S[:, :, 3, :] = o3
        S[:, :, :3, :] = o012.transpose(0, 1, 3, 2, 4).reshape(128, W32, 3, nq)
        S = S.reshape(128, NK, BL, 4, nq)
        o = S.transpose(2, 4, 3, 1, 0).reshape(BL, t_steps, U)
        outs.append(o)
    full = np.concatenate(outs, axis=0) * EPS
    if _trace:
        return full.astype(np.float32), res
    return full.astype(np.float32)


# revision 27
# speedup vs baseline: 1.1018x; 1.1018x over previous
"""Antisymmetric RNN kernel for Trainium2, data-parallel over batch on 8 cores.

Math (reference):
    M = W - W^T - gamma*I
    h_t = x_t @ V + bias                      [B, U]
    state_{t+1} = state_t + eps*tanh(h_t + state_t @ M)
    out[:, t] = state_{t+1}

Device formulation (per core, B_local=16), rescaled S' = state/eps,
M' = eps*M:
    S'_{t+1} = S'_t + tanh(h_t + S'_t @ M')

||M'|| is tiny (diag -1e-4, off-diag ~5e-7), so one Picard sweep over
the whole trajectory with a coarse (32-step piecewise-constant) state
estimate in the correction term converges:
    Sc  = x0/eps + prefix sums of 32-step block sums of h
          (tanh(h) ~ h there: the cubic error is zero-mean and enters
          z only through the 1e-4-scaled M')
    S   = x0/eps + cumsum(tanh(h + Sc@M'))   [exact, full res]
Measured rel err vs the exact recurrence: ~6e-3 (threshold 2e-2).

The 32-step block sums of x are computed on the host (input prep), so
sweep 0 on device is just 2 matmuls (xblk @ V) + 32 short prefix scans.
DVE's serial scan runs at ~2.4 cyc/element, so the full-res cumsum is
4-way de-interleaved: quad sums Q scan at T/4 resolution (f32), and
S[4k+c] are reconstructed with single adds off the f32 carries.

Layout: partitions carry u (2 chunks of 128); free dims are
(batch-outer, time-inner); th tile is [.., kc, 4] (par minor) so ACT
writes stay packed and parity slices are strided DVE reads. The coarse
Sc feeds PE matmuls via stride-0 broadcast APs (explicit dep edge -
broadcast reads are invisible to tile dep tracking).
"""

import sys

sys.path.insert(0, "/opt/trn_rl_repo")

import numpy as np
import ml_dtypes

import concourse.bass as bass
import concourse.bacc as bacc
import concourse.mybir as mybir
import concourse.tile as tile
from concourse.tile import add_dep_helper

EPS = 0.01
GAMMA = 0.01
B, T, D, U = 128, 1024, 128, 256
NCORES = 8
BL = B // NCORES  # 16 batch rows per core
NK = U // 128  # 2 u-chunks
W32 = NK * BL  # 32 (chunk, batch) columns
TCB = 128  # timesteps per PSUM tile / ACT instruction (4 banks)
QB = 4  # batch rows per matmul accumulation group (1 bank)
RB = 32  # coarse-S0 block size (piecewise-constant correction)

F32 = mybir.dt.float32
BF16 = mybir.dt.bfloat16
BF16_NP = ml_dtypes.bfloat16

_CACHED = {}


def build_nc(t_steps=T):
    nc = bacc.Bacc(None, target_bir_lowering=False)
    x_d = nc.declare_dram_parameter("xT", [D, BL, t_steps], BF16, isOutput=False)
    m_d = nc.declare_dram_parameter("Mp", [128, NK, NK, 128], BF16, isOutput=False)
    v_d = nc.declare_dram_parameter("Vp", [D, NK, 128], BF16, isOutput=False)
    b_d = nc.declare_dram_parameter("bT", [128, NK], F32, isOutput=False)
    x0_d = nc.declare_dram_parameter("x0T", [128, NK], F32, isOutput=False)
    x0b_d = nc.declare_dram_parameter("x0B", [128, W32], BF16, isOutput=False)
    x0f_d = nc.declare_dram_parameter("x0F", [128, W32], F32, isOutput=False)
    nb = t_steps // RB  # number of coarse blocks
    xb_d = nc.declare_dram_parameter("xB", [D, BL, nb], BF16, isOutput=False)
    nq = t_steps // 4
    ch1 = min(512, t_steps)
    n1 = t_steps // ch1
    kc = ch1 // 4
    o3_d = nc.declare_dram_parameter("out3", [128, W32, nq], F32, isOutput=True)
    o012_d = nc.declare_dram_parameter(
        "out012", [128, W32, n1, 3, kc], BF16, isOutput=True
    )

    Tanh = mybir.ActivationFunctionType.Tanh
    ADD = mybir.AluOpType.add
    BYPASS = mybir.AluOpType.bypass

    tcb = min(TCB, ch1)
    assert t_steps % ch1 == 0 and ch1 % tcb == 0 and tcb % RB == 0

    with tile.TileContext(nc) as tc:
        with (
            tc.tile_pool(name="const", bufs=1) as cpool,
            tc.tile_pool(name="xp", bufs=1) as xpool,
            tc.tile_pool(name="th", bufs=2) as thpool,
            tc.tile_pool(name="rec", bufs=1) as rpool,
            tc.tile_pool(name="ps", bufs=1, space=bass.MemorySpace.PSUM) as ppool,
        ):
            m_sb = cpool.tile([128, NK, NK, 128], BF16)
            v_sb = cpool.tile([D, NK, 128], BF16)
            b_sb = cpool.tile([128, NK], F32)
            x0_sb = cpool.tile([128, NK], F32)
            xb_sb = cpool.tile([D, BL, nb], BF16)
            # coarse prefix sums; slot 0 = x0/eps, slot m = prefix thru block m-1
            sc_sb = cpool.tile([128, W32, 1 + nb], BF16)
            # quad prefix sums (S at t=4k+3); slot 0 = x0/eps
            sq_sb = cpool.tile([128, W32, 1 + nq], F32)
            nc.sync.dma_start(xb_sb[:], xb_d[:])
            nc.sync.dma_start(v_sb[:], v_d[:])
            nc.sync.dma_start(m_sb[:], m_d[:])
            nc.sync.dma_start(b_sb[:], b_d[:])
            nc.sync.dma_start(x0_sb[:], x0_d[:])
            x0b_dma = nc.sync.dma_start(sc_sb[:, :, 0:1], x0b_d[:].unsqueeze(2))
            nc.sync.dma_start(sq_sb[:, :, 0:1], x0f_d[:].unsqueeze(2))

            x_sb = xpool.tile([D, BL, t_steps], BF16)
            nx = max(1, t_steps // 128)
            for c in range(nx):
                sl = slice(c * (t_steps // nx), (c + 1) * (t_steps // nx))
                nc.sync.dma_start(x_sb[:, :, sl], x_d[:, :, sl])

            # ---- sweep 0: coarse Sc: bsum = xblk@V (PE), short prefix scans
            # bsum psum borrows bank 0 of the z tiles (flat [b, m] layout)
            tot = BL * nb
            rows = max(1, tot // 128)
            cols = tot // rows
            bs_tiles = []
            for h in range(NK):
                z = ppool.tile([128, BL, tcb], F32, tag=f"z{h}")
                bs = z[:, 0:rows, 0:cols].rearrange(
                    "p a (b m) -> p (a b) m", m=nb
                )
                nc.tensor.matmul(bs, v_sb[:, h, :], xb_sb[:], start=True, stop=True)
                bs_tiles.append(z)
            dummy = b_sb[:, 0:1].broadcast_to([128, nb])
            sc_scans = [None] * W32
            for h in range(NK):
                z = bs_tiles[h]
                for b in range(BL):
                    j = h * BL + b
                    fi = b * nb
                    sc_scans[j] = nc.vector.tensor_tensor_scan(
                        sc_sb[:, j, 1 : 1 + nb],
                        z[:, fi // 128, fi % 128 : fi % 128 + nb],
                        dummy,
                        x0_sb[:, h : h + 1],
                        ADD,
                        BYPASS,
                    ).ins
            # broadcast-AP reads are invisible to tile dep tracking: first
            # block gets per-column edges onto the Sc scans (so PE starts as
            # soon as the columns it needs are ready), the next block one
            # coarse edge onto the last scan; PE program order covers the rest
            state = {"blk": 0}

            def emit_block(th_out, t0, h):
                # z = x@V + Sc@M' in PSUM; th_out = tanh(z + b)
                z = ppool.tile([128, BL, tcb], F32, tag=f"z{h}")
                m0 = t0 // RB  # first coarse block of this range
                nblk = tcb // RB
                blk_i = state["blk"]
                state["blk"] += 1
                for q in range(BL // QB):
                    zq = z[:, q * QB : (q + 1) * QB, :]
                    xq = x_sb[:, q * QB : (q + 1) * QB, t0 : t0 + tcb]
                    nc.tensor.matmul(zq, v_sb[:, h, :], xq, start=True, stop=False)
                    for k in range(NK):
                        sq = (
                            sc_sb[
                                :,
                                k * BL + q * QB : k * BL + (q + 1) * QB,
                                m0 : m0 + nblk,
                            ]
                            .unsqueeze(3)
                            .broadcast_to([128, QB, nblk, RB])
                        )
                        mm = nc.tensor.matmul(
                            zq, m_sb[:, k, h, :], sq, start=False, stop=(k == NK - 1)
                        )
                        if blk_i == 0:
                            add_dep_helper(
                                mm.ins,
                                sc_scans[k * BL + q * QB + QB - 1],
                                reason="Sc broadcast read",
                            )
                            if q == 0 and k == 0:
                                add_dep_helper(
                                    mm.ins, x0b_dma.ins, reason="Sc slot0 read"
                                )
                        elif blk_i == 1 and q == 0 and k == 0:
                            add_dep_helper(
                                mm.ins, sc_scans[W32 - 1], reason="Sc broadcast read"
                            )
                nc.scalar.activation(th_out, z[:], Tanh, bias=b_sb[:, h : h + 1])

            # ---- sweep 1: full-res S via 4-way de-interleaved cumsum ----
            # th1 tile is [128, W32, kc, 4]: same memory as packed time order
            # (par minor), so ACT writes stay contiguous and parity slices
            # are strided reads on DVE.
            for c in range(n1):
                th = thpool.tile([128, W32, kc, 4], BF16, tag="th1")
                for blk in range(ch1 // tcb):
                    t0 = c * ch1 + blk * tcb
                    k0 = blk * (tcb // 4)
                    for h in range(NK):
                        out_ap = th[
                            :, h * BL : (h + 1) * BL, k0 : k0 + tcb // 4, :
                        ].rearrange("p b k r -> p b (k r)")
                        emit_block(out_ap, t0, h)
                # prep: pair sums (strided reads, 1x) then quad sums (2x);
                p01 = rpool.tile([128, W32, kc], BF16, tag="p01")
                p23 = rpool.tile([128, W32, kc], BF16, tag="p23")
                qq = rpool.tile([128, W32, kc], BF16, tag="qq")
                nc.vector.tensor_tensor(p01[:], th[:, :, :, 0], th[:, :, :, 1], ADD)
                nc.vector.tensor_tensor(p23[:], th[:, :, :, 2], th[:, :, :, 3], ADD)
                nc.vector.tensor_tensor(qq[:], p01[:], p23[:], ADD)
                # scan quads (f32 state+out), chained through sq_sb slots
                for h in range(NK):
                    for b in range(BL):
                        j = h * BL + b
                        nc.vector.tensor_tensor_scan(
                            sq_sb[:, j, 1 + c * kc : 1 + (c + 1) * kc],
                            qq[:, j, :],
                            qq[:, j, :],
                            sq_sb[:, j, c * kc : c * kc + 1],
                            ADD,
                            BYPASS,
                        )
                nc.sync.dma_start(
                    o3_d[:, :, c * kc : (c + 1) * kc],
                    sq_sb[:, :, 1 + c * kc : 1 + (c + 1) * kc],
                )
                # reconstruct parities 0..2 off the f32 carries C:
                # S[4k]=C+q0, S[4k+1]=C+p01, S[4k+2]=S[4k+1]+q2
                recs = rpool.tile([128, W32, 3, kc], BF16, tag="recs")
                cq = sq_sb[:, :, c * kc : (c + 1) * kc]
                nc.vector.tensor_tensor(recs[:, :, 0, :], cq, th[:, :, :, 0], ADD)
                nc.vector.tensor_tensor(recs[:, :, 1, :], cq, p01[:], ADD)
                nc.vector.tensor_tensor(
                    recs[:, :, 2, :], recs[:, :, 1, :], th[:, :, :, 2], ADD
                )
                nc.sync.dma_start(o012_d[:, :, c, :, :], recs[:])

    nc.compile()
    return nc


def _prep_consts(V, W, bias, x0):
    M = W - W.T - GAMMA * np.eye(U, dtype=np.float32)
    Mp = (EPS * M).reshape(NK, 128, NK, 128).transpose(1, 0, 2, 3)
    Vp = V.reshape(D, NK, 128)
    bT = np.ascontiguousarray(bias.reshape(NK, 128).T)
    x0T = np.ascontiguousarray((x0 / EPS).reshape(NK, 128).T)
    x0B = np.repeat(x0T, BL, axis=1)  # [128, W32] broadcast per (chunk, batch)
    return {
        "Mp": np.ascontiguousarray(Mp).astype(BF16_NP),
        "Vp": np.ascontiguousarray(Vp).astype(BF16_NP),
        "bT": bT.astype(np.float32),
        "x0T": x0T.astype(np.float32),
        "x0B": np.ascontiguousarray(x0B).astype(BF16_NP),
        "x0F": np.ascontiguousarray(x0B).astype(np.float32),
    }


def _install_ntff_hook():
    # Register the axon NTFF profile hook if the image's antenv lacks it,
    # so trace=True can return exec_time_ns. Harmless if anything fails.
    import types

    try:
        import antenv.axon_hooks  # noqa: F401

        return
    except ImportError:
        pass
    try:
        import antenv
        from trn_agent_boot.trn_boot import _ntff_profile_via_ctypes

        mod = types.ModuleType("antenv.axon_hooks")
        _h = [None]
        mod.set_axon_ntff_profile_hook = lambda h: _h.__setitem__(0, h)
        mod.get_axon_ntff_profile_hook = lambda: _h[0]
        sys.modules["antenv.axon_hooks"] = mod
        antenv.axon_hooks = mod
        mod.set_axon_ntff_profile_hook(
            _ntff_profile_via_ctypes("/opt/axon/libaxon_pjrt.so")
        )
    except Exception:
        pass


def kernel(inputs, V, W, bias, x0, _t_steps=None, _trace=False):
    _install_ntff_hook()
    from concourse.bass_utils import run_bass_kernel_spmd

    inputs = np.asarray(inputs, dtype=np.float32)
    V = np.asarray(V, dtype=np.float32)
    W = np.asarray(W, dtype=np.float32)
    bias = np.asarray(bias, dtype=np.float32)
    x0 = np.asarray(x0, dtype=np.float32)

    t_steps = _t_steps or inputs.shape[1]
    key = t_steps
    if key not in _CACHED:
        _CACHED[key] = build_nc(t_steps)
    nc = _CACHED[key]

    consts = _prep_consts(V, W, bias, x0)
    nb = t_steps // RB
    in_maps = []
    for i in range(NCORES):
        shard = inputs[i * BL : (i + 1) * BL, :t_steps, :]  # [16, t, 128]
        xT = np.ascontiguousarray(shard.transpose(2, 0, 1)).astype(BF16_NP)
        xblk = shard.reshape(BL, nb, RB, D).sum(axis=2)  # [16, nb, 128] f32
        xB = np.ascontiguousarray(xblk.transpose(2, 0, 1)).astype(BF16_NP)
        in_maps.append({"xT": xT, "xB": xB, **consts})

    res = run_bass_kernel_spmd(nc, in_maps, list(range(NCORES)), trace=_trace)
    nq = t_steps // 4
    outs = []
    for i in range(NCORES):
        o3 = res.results[i]["out3"]  # [128, W32, nq] f32 (parity 3)
        o012 = res.results[i]["out012"].astype(np.float32)  # [128,W32,n1,3,kc]
        n1 = o012.shape[2]
        # S[p, j=(h,b), par, k] -> out[b, 4k+par, h*128+p]
        S = np.empty((128, W32, 4, nq), dtype=np.float32)
        S[:, :, 3, :] = o3
        S[:, :, :3, :] = o012.transpose(0, 1, 3, 2, 4).reshape(128, W32, 3, nq)
        S = S.reshape(128, NK, BL, 4, nq)
        o = S.transpose(2, 4, 3, 1, 0).reshape(BL, t_steps, U)
        outs.append(o)
    full = np.concatenate(outs, axis=0) * EPS
    if _trace:
        return full.astype(np.float32), res
    return full.astype(np.float32)


# revision 29
# speedup vs baseline: 1.1691x; 1.0611x over previous
"""Antisymmetric RNN kernel for Trainium2, data-parallel over batch on 8 cores.

Math (reference):
    M = W - W^T - gamma*I
    h_t = x_t @ V + bias                      [B, U]
    state_{t+1} = state_t + eps*tanh(h_t + state_t @ M)
    out[:, t] = state_{t+1}

Device formulation (per core, B_local=16), rescaled S' = state/eps,
M' = eps*M:
    S'_{t+1} = S'_t + tanh(h_t + S'_t @ M')

||M'|| is tiny (diag -1e-4, off-diag ~5e-7), so one Picard sweep over
the whole trajectory with a coarse (32-step piecewise-constant) state
estimate in the correction term converges:
    Sc  = x0/eps + prefix sums of 32-step block sums of h
          (tanh(h) ~ h there: the cubic error is zero-mean and enters
          z only through the 1e-4-scaled M')
    S   = x0/eps + cumsum(tanh(h + Sc@M'))   [exact, full res]
Measured rel err vs the exact recurrence: ~6e-3 (threshold 2e-2).

The 32-step block sums of x are computed on the host (input prep), so
sweep 0 on device is just 2 matmuls (xblk @ V) + 32 short prefix scans.
DVE's serial scan runs at ~2.4 cyc/element, so the full-res cumsum is
4-way de-interleaved: quad sums Q scan at T/4 resolution (f32), and
S[4k+c] are reconstructed with single adds off the f32 carries.

Layout: partitions carry u (2 chunks of 128); free dims are
(batch-outer, time-inner); th tile is [.., kc, 4] (par minor) so ACT
writes stay packed and parity slices are strided DVE reads. The coarse
Sc feeds PE matmuls via stride-0 broadcast APs (explicit dep edge -
broadcast reads are invisible to tile dep tracking).
"""

import sys

sys.path.insert(0, "/opt/trn_rl_repo")

import numpy as np
import ml_dtypes

import concourse.bass as bass
import concourse.bacc as bacc
import concourse.mybir as mybir
import concourse.tile as tile
from concourse.tile import add_dep_helper

EPS = 0.01
GAMMA = 0.01
B, T, D, U = 128, 1024, 128, 256
NCORES = 8
BL = B // NCORES  # 16 batch rows per core
NK = U // 128  # 2 u-chunks
W32 = NK * BL  # 32 (chunk, batch) columns
TCB = 128  # timesteps per PSUM tile / ACT instruction (4 banks)
QB = 4  # batch rows per matmul accumulation group (1 bank)
RB = 32  # coarse-S0 block size (piecewise-constant correction)

F32 = mybir.dt.float32
BF16 = mybir.dt.bfloat16
BF16_NP = ml_dtypes.bfloat16

_CACHED = {}


def build_nc(t_steps=T):
    nc = bacc.Bacc(None, target_bir_lowering=False)
    x_d = nc.declare_dram_parameter("xT", [D, BL, t_steps], BF16, isOutput=False)
    m_d = nc.declare_dram_parameter("Mp", [128, NK, NK, 128], BF16, isOutput=False)
    v_d = nc.declare_dram_parameter("Vp", [D, NK, 128], BF16, isOutput=False)
    b_d = nc.declare_dram_parameter("bT", [128, NK], F32, isOutput=False)
    x0_d = nc.declare_dram_parameter("x0T", [128, NK], F32, isOutput=False)
    x0b_d = nc.declare_dram_parameter("x0B", [128, W32], BF16, isOutput=False)
    x0f_d = nc.declare_dram_parameter("x0F", [128, W32], F32, isOutput=False)
    nb = t_steps // RB  # number of coarse blocks
    xb_d = nc.declare_dram_parameter("xB", [D, BL, nb], BF16, isOutput=False)
    nq = t_steps // 4
    ch1 = min(256, t_steps)
    n1 = t_steps // ch1
    kc = ch1 // 4
    o3_d = nc.declare_dram_parameter("out3", [128, W32, nq], F32, isOutput=True)
    o012_d = nc.declare_dram_parameter(
        "out012", [128, W32, n1, 3, kc], BF16, isOutput=True
    )

    Tanh = mybir.ActivationFunctionType.Tanh
    ADD = mybir.AluOpType.add
    BYPASS = mybir.AluOpType.bypass

    tcb = min(TCB, ch1)
    assert t_steps % ch1 == 0 and ch1 % tcb == 0 and tcb % RB == 0

    with tile.TileContext(nc) as tc:
        with (
            tc.tile_pool(name="const", bufs=1) as cpool,
            tc.tile_pool(name="xp", bufs=1) as xpool,
            tc.tile_pool(name="th", bufs=2) as thpool,
            tc.tile_pool(name="rec", bufs=1) as rpool,
            tc.tile_pool(name="ps", bufs=1, space=bass.MemorySpace.PSUM) as ppool,
        ):
            m_sb = cpool.tile([128, NK, NK, 128], BF16)
            v_sb = cpool.tile([D, NK, 128], BF16)
            b_sb = cpool.tile([128, NK], F32)
            x0_sb = cpool.tile([128, NK], F32)
            xb_sb = cpool.tile([D, BL, nb], BF16)
            # coarse prefix sums; slot 0 = x0/eps, slot m = prefix thru block m-1
            sc_sb = cpool.tile([128, W32, 1 + nb], BF16)
            # quad prefix sums (S at t=4k+3); slot 0 = x0/eps
            sq_sb = cpool.tile([128, W32, 1 + nq], F32)
            x_sb = xpool.tile([D, BL, t_steps], BF16)
            nx = max(1, t_steps // 128)
            xsz = t_steps // nx
            nc.sync.dma_start(xb_sb[:], xb_d[:])
            nc.sync.dma_start(v_sb[:], v_d[:])
            nc.sync.dma_start(x_sb[:, :, 0:xsz], x_d[:, :, 0:xsz])
            nc.sync.dma_start(m_sb[:], m_d[:])
            nc.sync.dma_start(b_sb[:], b_d[:])
            nc.sync.dma_start(x0_sb[:], x0_d[:])
            x0b_dma = nc.sync.dma_start(sc_sb[:, :, 0:1], x0b_d[:].unsqueeze(2))
            nc.sync.dma_start(sq_sb[:, :, 0:1], x0f_d[:].unsqueeze(2))
            for c in range(1, nx):
                sl = slice(c * xsz, (c + 1) * xsz)
                nc.sync.dma_start(x_sb[:, :, sl], x_d[:, :, sl])

            # ---- sweep 0: coarse Sc: bsum = xblk@V (PE), short prefix scans
            # bsum psum borrows bank 0 of the z tiles (flat [b, m] layout)
            tot = BL * nb
            rows = max(1, tot // 128)
            cols = tot // rows
            bss = cpool.tile([128, NK, BL, nb], F32)
            for h in range(NK):
                z = ppool.tile([128, BL, tcb], F32, tag=f"z{h}")
                bs = z[:, 0:rows, 0:cols].rearrange(
                    "p a (b m) -> p (a b) m", m=nb
                )
                bmm = nc.tensor.matmul(
                    bs, v_sb[:, h, :], xb_sb[:], start=True, stop=True
                )
                # evacuate to SBUF so the z tile is free for sweep 1 (the
                # rearranged psum AP is invisible to dep tracking: explicit
                # edge onto the matmul)
                cp = nc.scalar.copy(bss[:, h], bs)
                add_dep_helper(cp.ins, bmm.ins, reason="bs evacuate")
            dummy = b_sb[:, 0:1].broadcast_to([128, nb])
            sc_scans = [None] * W32
            for h in range(NK):
                for b in range(BL):
                    j = h * BL + b
                    sc_scans[j] = nc.vector.tensor_tensor_scan(
                        sc_sb[:, j, 1 : 1 + nb],
                        bss[:, h, b, :],
                        dummy,
                        x0_sb[:, h : h + 1],
                        ADD,
                        BYPASS,
                    ).ins
            # broadcast-AP reads are invisible to tile dep tracking: first
            # block gets per-column edges onto the Sc scans (so PE starts as
            # soon as the columns it needs are ready), the next block one
            # coarse edge onto the last scan; PE program order covers the rest
            state = {"blk": 0}

            def emit_block(th_out, t0, h):
                # z = x@V + Sc@M' in PSUM; th_out = tanh(z + b)
                z = ppool.tile([128, BL, tcb], F32, tag=f"z{h}")
                m0 = t0 // RB  # first coarse block of this range
                nblk = tcb // RB
                blk_i = state["blk"]
                state["blk"] += 1
                for q in range(BL // QB):
                    zq = z[:, q * QB : (q + 1) * QB, :]
                    xq = x_sb[:, q * QB : (q + 1) * QB, t0 : t0 + tcb]
                    nc.tensor.matmul(zq, v_sb[:, h, :], xq, start=True, stop=False)
                    for k in range(NK):
                        sq = (
                            sc_sb[
                                :,
                                k * BL + q * QB : k * BL + (q + 1) * QB,
                                m0 : m0 + nblk,
                            ]
                            .unsqueeze(3)
                            .broadcast_to([128, QB, nblk, RB])
                        )
                        mm = nc.tensor.matmul(
                            zq, m_sb[:, k, h, :], sq, start=False, stop=(k == NK - 1)
                        )
                        if blk_i == 0:
                            add_dep_helper(
                                mm.ins,
                                sc_scans[k * BL + q * QB + QB - 1],
                                reason="Sc broadcast read",
                            )
                            if q == 0 and k == 0:
                                add_dep_helper(
                                    mm.ins, x0b_dma.ins, reason="Sc slot0 read"
                                )
                        elif blk_i == 1 and q == 0 and k == 0:
                            add_dep_helper(
                                mm.ins, sc_scans[W32 - 1], reason="Sc broadcast read"
                            )
                nc.scalar.activation(th_out, z[:], Tanh, bias=b_sb[:, h : h + 1])

            # ---- sweep 1: full-res S via 4-way de-interleaved cumsum ----
            # th1 tile is [128, W32, kc, 4]: same memory as packed time order
            # (par minor), so ACT writes stay contiguous and parity slices
            # are strided reads on DVE.
            for c in range(n1):
                th = thpool.tile([128, W32, kc, 4], BF16, tag="th1")
                for blk in range(ch1 // tcb):
                    t0 = c * ch1 + blk * tcb
                    k0 = blk * (tcb // 4)
                    for h in range(NK):
                        out_ap = th[
                            :, h * BL : (h + 1) * BL, k0 : k0 + tcb // 4, :
                        ].rearrange("p b k r -> p b (k r)")
                        emit_block(out_ap, t0, h)
                # prep: pair sums (strided reads, 1x) then quad sums (2x);
                p01 = rpool.tile([128, W32, kc], BF16, tag="p01")
                p23 = rpool.tile([128, W32, kc], BF16, tag="p23")
                qq = rpool.tile([128, W32, kc], BF16, tag="qq")
                nc.vector.tensor_tensor(p01[:], th[:, :, :, 0], th[:, :, :, 1], ADD)
                nc.vector.tensor_tensor(p23[:], th[:, :, :, 2], th[:, :, :, 3], ADD)
                nc.vector.tensor_tensor(qq[:], p01[:], p23[:], ADD)
                # scan quads (f32 state+out) chained through sq_sb slots,
                # then reconstruct parities 0..2 off the f32 carries C
                # (S[4k]=C+q0, S[4k+1]=C+p01, S[4k+2]=S[4k+1]+q2) and DMA
                # out, per h-half so output drains early
                recs = rpool.tile([128, W32, 3, kc], BF16, tag="recs")
                for h in range(NK):
                    hs = slice(h * BL, (h + 1) * BL)
                    for b in range(BL):
                        j = h * BL + b
                        nc.vector.tensor_tensor_scan(
                            sq_sb[:, j, 1 + c * kc : 1 + (c + 1) * kc],
                            qq[:, j, :],
                            qq[:, j, :],
                            sq_sb[:, j, c * kc : c * kc + 1],
                            ADD,
                            BYPASS,
                        )
                    nc.sync.dma_start(
                        o3_d[:, hs, c * kc : (c + 1) * kc],
                        sq_sb[:, hs, 1 + c * kc : 1 + (c + 1) * kc],
                    )
                    cq = sq_sb[:, hs, c * kc : (c + 1) * kc]
                    nc.vector.tensor_tensor(
                        recs[:, hs, 0, :], cq, th[:, hs, :, 0], ADD
                    )
                    nc.vector.tensor_tensor(recs[:, hs, 1, :], cq, p01[:, hs, :], ADD)
                    nc.vector.tensor_tensor(
                        recs[:, hs, 2, :], recs[:, hs, 1, :], th[:, hs, :, 2], ADD
                    )
                    nc.sync.dma_start(o012_d[:, hs, c, :, :], recs[:, hs])

    nc.compile()
    return nc


def _prep_consts(V, W, bias, x0):
    M = W - W.T - GAMMA * np.eye(U, dtype=np.float32)
    Mp = (EPS * M).reshape(NK, 128, NK, 128).transpose(1, 0, 2, 3)
    Vp = V.reshape(D, NK, 128)
    bT = np.ascontiguousarray(bias.reshape(NK, 128).T)
    x0T = np.ascontiguousarray((x0 / EPS).reshape(NK, 128).T)
    x0B = np.repeat(x0T, BL, axis=1)  # [128, W32] broadcast per (chunk, batch)
    return {
        "Mp": np.ascontiguousarray(Mp).astype(BF16_NP),
        "Vp": np.ascontiguousarray(Vp).astype(BF16_NP),
        "bT": bT.astype(np.float32),
        "x0T": x0T.astype(np.float32),
        "x0B": np.ascontiguousarray(x0B).astype(BF16_NP),
        "x0F": np.ascontiguousarray(x0B).astype(np.float32),
    }


def _install_ntff_hook():
    # Register the axon NTFF profile hook if the image's antenv lacks it,
    # so trace=True can return exec_time_ns. Harmless if anything fails.
    import types

    try:
        import antenv.axon_hooks  # noqa: F401

        return
    except ImportError:
        pass
    try:
        import antenv
        from trn_agent_boot.trn_boot import _ntff_profile_via_ctypes

        mod = types.ModuleType("antenv.axon_hooks")
        _h = [None]
        mod.set_axon_ntff_profile_hook = lambda h: _h.__setitem__(0, h)
        mod.get_axon_ntff_profile_hook = lambda: _h[0]
        sys.modules["antenv.axon_hooks"] = mod
        antenv.axon_hooks = mod
        mod.set_axon_ntff_profile_hook(
            _ntff_profile_via_ctypes("/opt/axon/libaxon_pjrt.so")
        )
    except Exception:
        pass


def kernel(inputs, V, W, bias, x0, _t_steps=None, _trace=False):
    _install_ntff_hook()
    from concourse.bass_utils import run_bass_kernel_spmd

    inputs = np.asarray(inputs, dtype=np.float32)
    V = np.asarray(V, dtype=np.float32)
    W = np.asarray(W, dtype=np.float32)
    bias = np.asarray(bias, dtype=np.float32)
    x0 = np.asarray(x0, dtype=np.float32)

    t_steps = _t_steps or inputs.shape[1]
    key = t_steps
    if key not in _CACHED:
        _CACHED[key] = build_nc(t_steps)
    nc = _CACHED[key]

    consts = _prep_consts(V, W, bias, x0)
    nb = t_steps // RB
    in_maps = []
    for i in range(NCORES):
        shard = inputs[i * BL : (i + 1) * BL, :t_steps, :]  # [16, t, 128]
        xT = np.ascontiguousarray(shard.transpose(2, 0, 1)).astype(BF16_NP)
        xblk = shard.reshape(BL, nb, RB, D).sum(axis=2)  # [16, nb, 128] f32
        xB = np.ascontiguousarray(xblk.transpose(2, 0, 1)).astype(BF16_NP)
        in_maps.append({"xT": xT, "xB": xB, **consts})

    res = run_bass_kernel_spmd(nc, in_maps, list(range(NCORES)), trace=_trace)
    nq = t_steps // 4
    outs = []
    for i in range(NCORES):
        o3 = res.results[i]["out3"]  # [128, W32, nq] f32 (parity 3)
        o012 = res.results[i]["out012"].astype(np.float32)  # [128,W32,n1,3,kc]
        n1 = o012.shape[2]
        # S[p, j=(h,b), par, k] -> out[b, 4k+par, h*128+p]
        S = np.empty((128, W32, 4, nq), dtype=np.float32)
        S[:, :, 3, :] = o3
        S[:, :, :3, :] = o012.transpose(0, 1, 3, 2, 4).reshape(128, W32, 3, nq)
        S = S.reshape(128, NK, BL, 4, nq)
        o = S.transpose(2, 4, 3, 1, 0).reshape(BL, t_steps, U)
        outs.append(o)
    full = np.concatenate(outs, axis=0) * EPS
    if _trace:
        return full.astype(np.float32), res
    return full.astype(np.float32)


# revision 33
# speedup vs baseline: 1.2011x; 1.0273x over previous
"""Antisymmetric RNN kernel for Trainium2, data-parallel over batch on 8 cores.

Math (reference):
    M = W - W^T - gamma*I
    h_t = x_t @ V + bias                      [B, U]
    state_{t+1} = state_t + eps*tanh(h_t + state_t @ M)
    out[:, t] = state_{t+1}

Device formulation (per core, B_local=16), rescaled S' = state/eps,
M' = eps*M:
    S'_{t+1} = S'_t + tanh(h_t + S'_t @ M')

||M'|| is tiny (diag -1e-4, off-diag ~5e-7), so one Picard sweep over
the whole trajectory with a coarse (32-step piecewise-constant) state
estimate in the correction term converges:
    Sc  = x0/eps + prefix sums of 32-step block sums of h
          (tanh(h) ~ h there: the cubic error is zero-mean and enters
          z only through the 1e-4-scaled M')
    S   = x0/eps + cumsum(tanh(h + Sc@M'))   [exact, full res]
Measured rel err vs the exact recurrence: ~6e-3 (threshold 2e-2).

The 32-step block sums of x are computed on the host (input prep), so
sweep 0 on device is just 2 matmuls (xblk @ V) + 32 short prefix scans.
DVE's serial scan runs at ~2.4 cyc/element, so the full-res cumsum is
4-way de-interleaved: quad sums Q scan at T/4 resolution (f32), and
S[4k+c] are reconstructed with single adds off the f32 carries.

Layout: partitions carry u (2 chunks of 128); free dims are
(batch-outer, time-inner); th tile is [.., kc, 4] (par minor) so ACT
writes stay packed and parity slices are strided DVE reads. The coarse
Sc feeds PE matmuls via stride-0 broadcast APs (explicit dep edge -
broadcast reads are invisible to tile dep tracking).
"""

import sys

sys.path.insert(0, "/opt/trn_rl_repo")

import numpy as np
import ml_dtypes

import concourse.bass as bass
import concourse.bacc as bacc
import concourse.mybir as mybir
import concourse.tile as tile
from concourse.tile import add_dep_helper

EPS = 0.01
GAMMA = 0.01
B, T, D, U = 128, 1024, 128, 256
NCORES = 8
BL = B // NCORES  # 16 batch rows per core
NK = U // 128  # 2 u-chunks
W32 = NK * BL  # 32 (chunk, batch) columns
TCB = 128  # timesteps per PSUM tile / ACT instruction (4 banks)
QB = 4  # batch rows per matmul accumulation group (1 bank)
RB = 32  # coarse-S0 block size (piecewise-constant correction)

F32 = mybir.dt.float32
BF16 = mybir.dt.bfloat16
BF16_NP = ml_dtypes.bfloat16

_CACHED = {}


def build_nc(t_steps=T):
    nc = bacc.Bacc(None, target_bir_lowering=False)
    x_d = nc.declare_dram_parameter("xT", [D, BL, t_steps // 4, 4], BF16, isOutput=False)
    m_d = nc.declare_dram_parameter("Mp", [128, NK, NK, 128], BF16, isOutput=False)
    v_d = nc.declare_dram_parameter("Vp", [D, NK, 128], BF16, isOutput=False)
    b_d = nc.declare_dram_parameter("bT", [128, NK], F32, isOutput=False)
    x0_d = nc.declare_dram_parameter("x0T", [128, NK], F32, isOutput=False)
    x0b_d = nc.declare_dram_parameter("x0B", [128, W32], BF16, isOutput=False)
    x0f_d = nc.declare_dram_parameter("x0F", [128, W32], F32, isOutput=False)
    nb = t_steps // RB  # number of coarse blocks
    xb_d = nc.declare_dram_parameter("xB", [D, BL, nb], BF16, isOutput=False)
    nq = t_steps // 4
    ch1 = min(256, t_steps)
    n1 = t_steps // ch1
    kc = ch1 // 4
    o3_d = nc.declare_dram_parameter("out3", [128, W32, nq], F32, isOutput=True)
    o012_d = nc.declare_dram_parameter(
        "out012", [128, W32, n1, 3, kc], BF16, isOutput=True
    )

    Tanh = mybir.ActivationFunctionType.Tanh
    ADD = mybir.AluOpType.add
    BYPASS = mybir.AluOpType.bypass

    tcb = min(TCB, ch1)
    assert t_steps % ch1 == 0 and ch1 % tcb == 0 and tcb % RB == 0

    with tile.TileContext(nc) as tc:
        with (
            tc.tile_pool(name="const", bufs=1) as cpool,
            tc.tile_pool(name="xp", bufs=1) as xpool,
            tc.tile_pool(name="th", bufs=2) as thpool,
            tc.tile_pool(name="rec", bufs=1) as rpool,
            tc.tile_pool(name="ps", bufs=1, space=bass.MemorySpace.PSUM) as ppool,
        ):
            m_sb = cpool.tile([128, NK, NK, 128], BF16)
            v_sb = cpool.tile([D, NK, 128], BF16)
            b_sb = cpool.tile([128, NK], F32)
            x0_sb = cpool.tile([128, NK], F32)
            xb_sb = cpool.tile([D, BL, nb], BF16)
            # coarse prefix sums; slot 0 = x0/eps, slot m = prefix thru block m-1
            sc_sb = cpool.tile([128, W32, 1 + nb], BF16)
            # quad prefix sums (S at t=4k+3); slot 0 = x0/eps
            sq_sb = cpool.tile([128, W32, 1 + nq], F32)
            x_sb = xpool.tile([D, BL, t_steps // 4, 4], BF16)
            nx = max(1, t_steps // 128)
            xsz = t_steps // nx // 4
            nc.sync.dma_start(xb_sb[:], xb_d[:])
            nc.sync.dma_start(v_sb[:], v_d[:])
            nc.sync.dma_start(x_sb[:, :, 0:xsz, :], x_d[:, :, 0:xsz, :])
            nc.sync.dma_start(m_sb[:], m_d[:])
            nc.sync.dma_start(b_sb[:], b_d[:])
            nc.sync.dma_start(x0_sb[:], x0_d[:])
            x0b_dma = nc.sync.dma_start(sc_sb[:, :, 0:1], x0b_d[:].unsqueeze(2))
            nc.sync.dma_start(sq_sb[:, :, 0:1], x0f_d[:].unsqueeze(2))
            for c in range(1, nx):
                sl = slice(c * xsz, (c + 1) * xsz)
                nc.sync.dma_start(x_sb[:, :, sl, :], x_d[:, :, sl, :])

            # ---- sweep 0: coarse Sc: bsum = xblk@V (PE), short prefix scans
            # bsum psum borrows bank 0 of the z tiles (flat [b, m] layout)
            tot = BL * nb
            rows = max(1, tot // 128)
            bss = cpool.tile([128, NK, BL, nb], F32)
            bs_cp = []
            for h in range(NK):
                z = ppool.tile([128, BL, tcb // 4, 4], F32, tag=f"z{h}")
                bs = z[:, 0:rows, 0 : tot // rows // 4, :]
                bmm = nc.tensor.matmul(
                    bs, v_sb[:, h, :], xb_sb[:], start=True, stop=True
                )
                # evacuate to SBUF so the z tile is free for sweep 1 (the
                # rearranged psum AP is invisible to dep tracking: explicit
                # edge onto the matmul)
                cp = nc.scalar.copy(
                    bss[:, h],
                    bs.rearrange(
                        "p a (b2 kk) r -> p (a b2) (kk r)", b2=(BL // rows) // 4
                    )
                    if rows * 4 <= BL
                    else bs.rearrange("p a k r -> p a (k r)"),
                )
                add_dep_helper(cp.ins, bmm.ins, reason="bs evacuate")
                bs_cp.append(cp.ins)
            dummy = b_sb[:, 0:1].broadcast_to([128, nb])
            sc_scans = [None] * W32
            for h in range(NK):
                for b in range(BL):
                    j = h * BL + b
                    sc_scans[j] = nc.vector.tensor_tensor_scan(
                        sc_sb[:, j, 1 : 1 + nb],
                        bss[:, h, b, :],
                        dummy,
                        x0_sb[:, h : h + 1],
                        ADD,
                        BYPASS,
                    ).ins
            # broadcast-AP reads are invisible to tile dep tracking: first
            # block gets per-column edges onto the Sc scans (so PE starts as
            # soon as the columns it needs are ready), the next block one
            # coarse edge onto the last scan; PE program order covers the rest
            state = {"blk": 0}

            def emit_block(th_out, t0, h):
                # z = x@V + Sc@M' in PSUM; th_out = tanh(z + b)
                z = ppool.tile([128, BL, tcb // 4, 4], F32, tag=f"z{h}")
                m0 = t0 // RB  # first coarse block of this range
                nblk = tcb // RB
                blk_i = state["blk"]
                state["blk"] += 1
                for q in range(BL // QB):
                    zq = z[:, q * QB : (q + 1) * QB, :, :]
                    xq = x_sb[:, q * QB : (q + 1) * QB, t0 // 4 : (t0 + tcb) // 4, :]
                    xmm = nc.tensor.matmul(
                        zq, v_sb[:, h, :], xq, start=True, stop=False
                    )
                    if blk_i < NK and q == 0:
                        # WAR: don't overwrite the bs psum before its ACT
                        # copy (read via untracked rearranged AP) drains
                        add_dep_helper(xmm.ins, bs_cp[h], reason="bs WAR")
                    for k in range(NK):
                        sq = (
                            sc_sb[
                                :,
                                k * BL + q * QB : k * BL + (q + 1) * QB,
                                m0 : m0 + nblk,
                            ]
                            .unsqueeze(3)
                            .broadcast_to([128, QB, nblk, RB])
                        )
                        mm = nc.tensor.matmul(
                            zq, m_sb[:, k, h, :], sq, start=False, stop=(k == NK - 1)
                        )
                        if blk_i < NK:
                            # first t-block per h: per-column edges so PE can
                            # start as soon as the needed scans land
                            add_dep_helper(
                                mm.ins,
                                sc_scans[k * BL + q * QB + QB - 1],
                                reason="Sc broadcast read",
                            )
                            if q == 0 and k == 0:
                                add_dep_helper(
                                    mm.ins, x0b_dma.ins, reason="Sc slot0 read"
                                )
                        elif q == 0 and k == 0:
                            # broadcast reads are invisible to dep tracking
                            # and the scheduler may reorder unlinked same-
                            # engine instructions: every block needs an edge
                            add_dep_helper(
                                mm.ins, sc_scans[W32 - 1], reason="Sc broadcast read"
                            )
                return nc.scalar.activation(
                    th_out, z[:, :, :, :], Tanh, bias=b_sb[:, h : h + 1]
                )

            # ---- sweep 1: full-res S via 4-way de-interleaved cumsum ----
            # th1 tile is [128, W32, kc, 4]: same memory as packed time order
            # (par minor), so ACT writes stay contiguous and parity slices
            # are strided reads on DVE.
            rec2_insts = {}
            for c in range(n1):
                th = thpool.tile([128, W32, kc, 4], BF16, tag="th1")
                acts = []
                for blk in range(ch1 // tcb):
                    t0 = c * ch1 + blk * tcb
                    k0 = blk * (tcb // 4)
                    for h in range(NK):
                        out_ap = th[:, h * BL : (h + 1) * BL, k0 : k0 + tcb // 4, :]
                        acts.append(emit_block(out_ap, t0, h))
                if c >= 2:
                    # WAR: th rotates with 2 bufs; don't overwrite before
                    # chunk c-2's last strided reader is done
                    add_dep_helper(acts[0].ins, rec2_insts[c - 2], reason="th WAR")
                # prep: pair sums (strided reads, 1x) then quad sums (2x);
                p01 = rpool.tile([128, W32, kc], BF16, tag="p01")
                p23 = rpool.tile([128, W32, kc], BF16, tag="p23")
                qq = rpool.tile([128, W32, kc], BF16, tag="qq")
                # ACT wrote th via rearranged APs (invisible to dep
                # tracking): explicit edges onto the chunk's first DVE read;
                # DVE program order covers the rest
                i01 = nc.vector.tensor_tensor(
                    p01[:], th[:, :, :, 0], th[:, :, :, 1], ADD
                )
                for a in acts:
                    add_dep_helper(i01.ins, a.ins, reason="th ready")
                nc.vector.tensor_tensor(p23[:], th[:, :, :, 2], th[:, :, :, 3], ADD)
                nc.vector.tensor_tensor(qq[:], p01[:], p23[:], ADD)
                # scan quads (f32 state+out) chained through sq_sb slots,
                # then reconstruct parities 0..2 off the f32 carries C
                # (S[4k]=C+q0, S[4k+1]=C+p01, S[4k+2]=S[4k+1]+q2) and DMA
                # out, per h-half so output drains early
                recs = rpool.tile([128, W32, 3, kc], BF16, tag="recs")
                for h in range(NK):
                    hs = slice(h * BL, (h + 1) * BL)
                    for b in range(BL):
                        j = h * BL + b
                        nc.vector.tensor_tensor_scan(
                            sq_sb[:, j, 1 + c * kc : 1 + (c + 1) * kc],
                            qq[:, j, :],
                            qq[:, j, :],
                            sq_sb[:, j, c * kc : c * kc + 1],
                            ADD,
                            BYPASS,
                        )
                    nc.sync.dma_start(
                        o3_d[:, hs, c * kc : (c + 1) * kc],
                        sq_sb[:, hs, 1 + c * kc : 1 + (c + 1) * kc],
                    )
                    cq = sq_sb[:, hs, c * kc : (c + 1) * kc]
                    nc.vector.tensor_tensor(
                        recs[:, hs, 0, :], cq, th[:, hs, :, 0], ADD
                    )
                    nc.vector.tensor_tensor(recs[:, hs, 1, :], cq, p01[:, hs, :], ADD)
                    r2 = nc.vector.tensor_tensor(
                        recs[:, hs, 2, :], recs[:, hs, 1, :], th[:, hs, :, 2], ADD
                    )
                    rec2_insts[c] = r2.ins
                    nc.sync.dma_start(o012_d[:, hs, c, :, :], recs[:, hs])

    nc.compile()
    return nc


def _prep_consts(V, W, bias, x0):
    M = W - W.T - GAMMA * np.eye(U, dtype=np.float32)
    Mp = (EPS * M).reshape(NK, 128, NK, 128).transpose(1, 0, 2, 3)
    Vp = V.reshape(D, NK, 128)
    bT = np.ascontiguousarray(bias.reshape(NK, 128).T)
    x0T = np.ascontiguousarray((x0 / EPS).reshape(NK, 128).T)
    x0B = np.repeat(x0T, BL, axis=1)  # [128, W32] broadcast per (chunk, batch)
    return {
        "Mp": np.ascontiguousarray(Mp).astype(BF16_NP),
        "Vp": np.ascontiguousarray(Vp).astype(BF16_NP),
        "bT": bT.astype(np.float32),
        "x0T": x0T.astype(np.float32),
        "x0B": np.ascontiguousarray(x0B).astype(BF16_NP),
        "x0F": np.ascontiguousarray(x0B).astype(np.float32),
    }


def _install_ntff_hook():
    # Register the axon NTFF profile hook if the image's antenv lacks it,
    # so trace=True can return exec_time_ns. Harmless if anything fails.
    import types

    try:
        import antenv.axon_hooks  # noqa: F401

        return
    except ImportError:
        pass
    try:
        import antenv
        from trn_agent_boot.trn_boot import _ntff_profile_via_ctypes

        mod = types.ModuleType("antenv.axon_hooks")
        _h = [None]
        mod.set_axon_ntff_profile_hook = lambda h: _h.__setitem__(0, h)
        mod.get_axon_ntff_profile_hook = lambda: _h[0]
        sys.modules["antenv.axon_hooks"] = mod
        antenv.axon_hooks = mod
        mod.set_axon_ntff_profile_hook(
            _ntff_profile_via_ctypes("/opt/axon/libaxon_pjrt.so")
        )
    except Exception:
        pass


def kernel(inputs, V, W, bias, x0, _t_steps=None, _trace=False):
    _install_ntff_hook()
    from concourse.bass_utils import run_bass_kernel_spmd

    inputs = np.asarray(inputs, dtype=np.float32)
    V = np.asarray(V, dtype=np.float32)
    W = np.asarray(W, dtype=np.float32)
    bias = np.asarray(bias, dtype=np.float32)
    x0 = np.asarray(x0, dtype=np.float32)

    t_steps = _t_steps or inputs.shape[1]
    key = t_steps
    if key not in _CACHED:
        _CACHED[key] = build_nc(t_steps)
    nc = _CACHED[key]

    consts = _prep_consts(V, W, bias, x0)
    nb = t_steps // RB
    in_maps = []
    for i in range(NCORES):
        shard = inputs[i * BL : (i + 1) * BL, :t_steps, :]  # [16, t, 128]
        xT = np.ascontiguousarray(shard.transpose(2, 0, 1)).astype(BF16_NP)
        xblk = shard.reshape(BL, nb, RB, D).sum(axis=2)  # [16, nb, 128] f32
        xB = np.ascontiguousarray(xblk.transpose(2, 0, 1)).astype(BF16_NP)
        in_maps.append({"xT": xT, "xB": xB, **consts})

    res = run_bass_kernel_spmd(nc, in_maps, list(range(NCORES)), trace=_trace)
    nq = t_steps // 4
    outs = []
    for i in range(NCORES):
        o3 = res.results[i]["out3"]  # [128, W32, nq] f32 (parity 3)
        o012 = res.results[i]["out012"].astype(np.float32)  # [128,W32,n1,3,kc]
        n1 = o012.shape[2]
        # S[p, j=(h,b), par, k] -> out[b, 4k+par, h*128+p]
        S = np.empty((128, W32, 4, nq), dtype=np.float32)
        S[:, :, 3, :] = o3
        S[:, :, :3, :] = o012.transpose(0, 1, 3, 2, 4).reshape(128, W32, 3, nq)
        S = S.reshape(128, NK, BL, 4, nq)
        o = S.transpose(2, 4, 3, 1, 0).reshape(BL, t_steps, U)
        outs.append(o)
    full = np.concatenate(outs, axis=0) * EPS
    if _trace:
        return full.astype(np.float32), res
    return full.astype(np.float32)
